# revision 9
# baseline (speedup 1.0000x reference)
"""Multi-head attention TRN2 kernel (B=4, S=2048, D=1024, H=16).

Sharding: 8 cores = (batch, head-half). Core c handles batch c//2 and
heads (c%2)*8..+8 (projection dims (c%2)*512..+512) for ALL 2048
queries. Each core emits a partial O-projection output [2048, 1024];
the host sums the two partials per batch.

Key compression (exact): the mask is binary; masked keys get -1e9 added
to their logits, so exp underflows to exactly 0 in f32 -- they contribute
nothing. The host gathers only unmasked keys (~1004-1052 of 2048 here)
and pads to a multiple of 384 with -1e9 mask slots.

All matmul operands are bf16 (PSUM accumulation stays f32). kt and qt
are both bf16 too: the scores pair is row-tiled (tile_position) and at
bf16 both row-groups stream concurrently at 1 col/cycle (the old f32r
pair serialized on the 256B/cycle XBUS).

Per-core dataflow (contraction dim on SBUF partitions, PE computes
C[M,N] = lhsT[K,M].T @ rhs[K,N]):

  DMA prologue: every input DMA is submitted up front, in priority
  order, spread over 4 hardware queues (sync: xk slabs; scalar:
  m/vo/wk/wq/wo; gpsimd: xv slabs; vector: wv/xq0) so each phase's
  inputs land just before it starts.

  A:  KT[dout, k]  = wk.T-chunks x XkT
  B:  V[k, dh]     = XkT-chunks x wv       (head-strided [k, 8*(DH+1)]
                     with a ones column per head -> softmax denominator)
  B2: QT[dout, q]  for q-block 0 only
  C:  per q-block (512 q) x head pair pr (row-packed in the PE):
        for kc: scoresT[k,q] pair via two CONCURRENT row-tiled matmuls;
                PT = exp(scoresT + mask[kc])   (ACT bias = mask column);
                po[65, q] += (V_h | 1).T x PT  (PSUM accum over kc);
                then up to 2 FILLER matmuls: Q-projection of q-block
                qb+1 and O-projection of q-block qb-1 run inside the
                ACT-bound gaps, keeping the PE dense (HAM stays 2.4GHz).
      normalize: the PSUM pair is drained on TWO engines at once
                (po halves split gpsimd/vector), one wide
                reciprocal_approx_fast (DVE) covers both heads ->
                partition_broadcast (GPSIMD) -> DVE mul -> OT slab.
  D:  O-projection of the last q-block, staggered: each unit's first
      ndc-1 PSUM-accumulating matmuls depend only on earlier head
      pairs, so they run while the last normalize chain completes; the
      final matmul + drain copies/DMAs rotate across engines/queues.

  The attention inner loop is ~balanced between the PE and ACT(exp)
  (~1.0-1.1us per 1024-wide exp). Softmax max-subtraction is skipped:
  scores ~ N(0,1) here, exp is safe.
"""

import numpy as np
import ml_dtypes

import concourse.bass as bass
import concourse.bacc as bacc
import concourse.mybir as mybir
import concourse.tile as tile
from concourse.bass_utils import run_bass_kernel_spmd

F32 = mybir.dt.float32
F32R = mybir.dt.float32r
BF16 = mybir.dt.bfloat16
USE_BF16 = True
MDT = BF16 if USE_BF16 else F32R
NP_BF16 = np.dtype(ml_dtypes.bfloat16) if USE_BF16 else np.dtype(np.float32)

B, S, D, H = 4, 2048, 1024, 16
DH = D // H
P = 128
NCORES = 8
HPC = H // 2          # heads per core
DPC = D // 2          # projection dims per core
QBLK = 512


def build_nc(nkc, with_bias=False):
    """Per-core Bass program; nkc = number of 128-key chunks kept."""
    d, s, qblk, dh, hpc, dpc = D, S, QBLK, DH, HPC, DPC
    ndc = dpc // P        # output-dim chunks for this core (4)
    nd = d // P           # full-d contraction chunks (8)
    nqb = s // qblk       # q blocks (4)
    sk = nkc * P          # padded key count
    ksl = 384             # K/V-proj moving slab width
    nsl = sk // ksl
    assert nsl * ksl == sk, (sk, ksl)
    Exp = mybir.ActivationFunctionType.Exp

    mdt = MDT
    nc = bacc.Bacc()
    # all inputs host-packed so every DMA moves 6-8KB contiguous
    # per-partition lines: X tensors are [128, slab, chunk, width] with the
    # partition index innermost of the original d/row dim; weights are
    # [128, chunk*cols].
    xqt_d = nc.dram_tensor("xqt", [P, s // 512, nd, 512], mdt, kind="ExternalInput")
    xkt_d = nc.dram_tensor("xkt", [P, nsl, nd, ksl], mdt, kind="ExternalInput")
    xvt_d = nc.dram_tensor("xvt", [P, nsl, nd, ksl], mdt, kind="ExternalInput")
    wq_d = nc.dram_tensor("wq", [P, nd * dpc], mdt, kind="ExternalInput")
    wk_d = nc.dram_tensor("wk", [P, nd * dpc], mdt, kind="ExternalInput")
    wv_d = nc.dram_tensor("wv", [P, nd * dpc], mdt, kind="ExternalInput")
    wo_d = nc.dram_tensor("wo", [P, ndc * d], mdt, kind="ExternalInput")
    m_d = nc.dram_tensor("mrow", [P, nkc], F32, kind="ExternalInput")
    vones_d = nc.dram_tensor("vones", [P, hpc], mdt, kind="ExternalInput")
    if with_bias:
        ones_d = nc.dram_tensor("ones", [1, 512], mdt, kind="ExternalInput")
        bias_d = nc.dram_tensor("biases", [1, 3 * dpc], mdt, kind="ExternalInput")
    out_d = nc.dram_tensor("out", [s, d], F32, kind="ExternalOutput")

    mm = nc.tensor.matmul

    def ecopy(eng, out, in_):
        if eng is nc.scalar:
            eng.copy(out, in_)
        else:
            eng.tensor_copy(out, in_)

    with tile.TileContext(nc) as tc:
        with (
            tc.tile_pool(name="persist", bufs=1) as pp,
            tc.tile_pool(name="small", bufs=1) as sp,
            tc.tile_pool(name="xqp", bufs=2) as xqp,
            tc.tile_pool(name="xsp", bufs=3) as xsp,
            tc.tile_pool(name="xvp", bufs=3) as xvp,
        ):
            m_sb = sp.tile([P, nkc], F32, tag="m")
            if with_bias:
                ones_sb = sp.tile([1, 512], mdt, tag="ones")
                bias_sb = sp.tile([1, 3 * dpc], mdt, tag="bias")

            # kt/qt both bf16: the bf16 row-tiled pair streams both row
            # groups concurrently (f32r pairs serialize on the XBUS).
            sdt = mdt
            kt_t = [pp.tile([P, sk], sdt, tag=f"kt{i}", name=f"kt{i}") for i in range(ndc)]
            v_t = [pp.tile([P, hpc * (dh + 1)], mdt, tag=f"v{i}", name=f"v{i}")
                   for i in range(nkc)]
            qt_t = [pp.tile([P, s], sdt, tag=f"qt{i}", name=f"qt{i}") for i in range(ndc)]
            ot_t = [pp.tile([P, s], mdt, tag=f"ot{i}", name=f"ot{i}") for i in range(ndc)]
            wq_t = pp.tile([P, nd, dpc], mdt, tag="wq", name="wq")
            wk_t = pp.tile([P, nd, dpc], mdt, tag="wk", name="wk")
            wv_t = pp.tile([P, nd, dpc], mdt, tag="wv", name="wv")
            wo_t = pp.tile([P, ndc, d], mdt, tag="wo", name="wo")
            vo_sb = sp.tile([P, hpc, 1], mdt, tag="vo")
            xk_sl = [xsp.tile([P, nd, ksl], mdt, tag="xk", name=f"xk{i}")
                     for i in range(nsl)]
            xv_sl = [xvp.tile([P, nd, ksl], mdt, tag="xv", name=f"xv{i}")
                     for i in range(nsl)]
            xq_sl0 = xqp.tile([P, nd, qblk], mdt, tag="xq", name="xq0")

            # ---- DMA prologue: everything submitted now, in priority ----
            # sync: xk slabs (phase A); scalar: small + wk,wq,wo;
            # gpsimd: xv slabs (phase B); vector: wv + xq slab 0.
            nc.scalar.dma_start(m_sb[:, :], m_d[:, :])
            nc.scalar.dma_start(vo_sb[:, :, :], vones_d[:, :, None])
            if with_bias:
                nc.scalar.dma_start(ones_sb[:, :], ones_d[:, :])
                nc.scalar.dma_start(bias_sb[:, :], bias_d[:, :])
            for i in range(nsl):
                nc.sync.dma_start(xk_sl[i][:, :, :], xkt_d[:, i, :, :])
            nc.scalar.dma_start(wk_t[:, :, :], wk_d[:, :].rearrange("p (c n) -> p c n", c=nd))
            nc.gpsimd.dma_start(wv_t[:, :, :], wv_d[:, :].rearrange("p (c n) -> p c n", c=nd))
            for i in range(nsl):
                nc.gpsimd.dma_start(xv_sl[i][:, :, :], xvt_d[:, i, :, :])
            nc.sync.dma_start(xq_sl0[:, :, :], xqt_d[:, 0, :, :])
            nc.scalar.dma_start(wq_t[:, :, :], wq_d[:, :].rearrange("p (c n) -> p c n", c=nd))
            nc.scalar.dma_start(wo_t[:, :, :], wo_d[:, :].rearrange("p (c n) -> p c n", c=ndc))

            # ---------------- phase A: K projection ----------------
            with tc.tile_pool(name="psA", bufs=4, space="PSUM") as psA:
                for ks in range(nsl):
                    for dc in range(ndc):
                        ps = psA.tile([P, ksl], F32, tag="ps")
                        for di in range(nd):
                            mm(ps[:, :], wk_t[:, di, dc * P:(dc + 1) * P],
                               xk_sl[ks][:, di, :],
                               start=(di == 0), stop=(di == nd - 1 and not with_bias))
                        if with_bias:
                            mm(ps[:, :], bias_sb[0:1, dpc + dc * P:dpc + (dc + 1) * P],
                               ones_sb[0:1, 0:ksl], start=False, stop=True)
                        nc.scalar.copy(kt_t[dc][:, ks * ksl:(ks + 1) * ksl], ps[:, :])

            # ---------------- phase B: V projection ----------------
            with tc.tile_pool(name="psB", bufs=4, space="PSUM") as psB:
                kc_per_slab = ksl // P
                for vsl in range(nsl):
                    for kci in range(kc_per_slab):
                        kc = vsl * kc_per_slab + kci
                        vt3 = v_t[kc].rearrange("p (g c) -> p g c", c=dh + 1)
                        nc.vector.tensor_copy(vt3[:, :, dh:dh + 1], vo_sb[:, :, :])
                        ps = psB.tile([P, dpc], F32, tag="ps")
                        for di in range(nd):
                            mm(ps[:, :], xv_sl[vsl][:, di, kci * P:(kci + 1) * P],
                               wv_t[:, di, :],
                               start=(di == 0), stop=(di == nd - 1 and not with_bias))
                        if with_bias:
                            mm(ps[:, :], ones_sb[0:1, 0:P],
                               bias_sb[0:1, 2 * dpc:3 * dpc], start=False, stop=True)
                        nc.scalar.copy(
                            vt3[:, :, 0:dh],
                            ps[:, :].rearrange("p (g c) -> p g c", c=dh),
                        )

            def dma_xq(iqb, sl):
                nc.sync.dma_start(sl[:, :, :], xqt_d[:, iqb, :, :])

            # ------- phase C: attention + interleaved Q/O-proj fillers -------
            with (
                tc.tile_pool(name="obp", bufs=3) as obp,
                tc.tile_pool(name="ptp", bufs=3) as ptp,
                tc.tile_pool(name="rcp", bufs=2) as rcp,
                tc.tile_pool(name="pbp", bufs=2) as pbp,
                tc.tile_pool(name="pss", bufs=2, space="PSUM") as pss,
                tc.tile_pool(name="pso", bufs=2, space="PSUM") as pso,
                tc.tile_pool(name="psf", bufs=2, space="PSUM") as psf,
            ):
                ocnt = [0]

                def q_unit(iqb, dc, sl):
                    """Q-projection of q-block iqb, dim chunk dc: 8 mms + copy."""
                    ps = [None]

                    def mk(di):
                        def op():
                            if di == 0:
                                ps[0] = psf.tile([P, qblk], F32, tag="f",
                                                 name=f"fq{iqb}_{dc}")
                            mm(ps[0][:, :], wq_t[:, di, dc * P:(dc + 1) * P],
                               sl[:, di, :], start=(di == 0),
                               stop=(di == nd - 1 and not with_bias))
                            if di == nd - 1:
                                if with_bias:
                                    mm(ps[0][:, :], bias_sb[0:1, dc * P:(dc + 1) * P],
                                       ones_sb[0:1, 0:qblk], start=False, stop=True)
                                nc.vector.tensor_copy(
                                    qt_t[dc][:, iqb * qblk:(iqb + 1) * qblk], ps[0][:, :])
                        return op
                    return [mk(di) for di in range(nd)]

                def o_unit(qc, nh):
                    """O-projection rows qc*128..+128, cols nh*512..+512."""
                    ps = [None]

                    def mk(dc):
                        def op():
                            if dc == 0:
                                ps[0] = psf.tile([P, 512], F32, tag="f",
                                                 name=f"fo{qc}_{nh}")
                            mm(ps[0][:, :], ot_t[dc][:, qc * P:(qc + 1) * P],
                               wo_t[:, dc, nh * 512:(nh + 1) * 512],
                               start=(dc == 0), stop=(dc == ndc - 1))
                            if dc == ndc - 1:
                                ob = obp.tile([P, 512], F32, tag="ob",
                                              name=f"ob{qc}_{nh}")
                                nc.vector.tensor_copy(ob[:, :], ps[0][:, :])
                                nc.sync.dma_start(
                                    out_d[qc * P:(qc + 1) * P,
                                          nh * 512:(nh + 1) * 512], ob[:, :])
                        return op
                    return [mk(dc) for dc in range(ndc)]

                for op in q_unit(0, 0, xq_sl0):
                    op()

                for iqb in range(nqb):
                    q0 = iqb * qblk
                    fillers = []
                    if iqb == 0:
                        for dc in range(1, ndc):
                            fillers += q_unit(0, dc, xq_sl0)
                    if iqb + 1 < nqb:
                        sl = xqp.tile([P, nd, qblk], mdt, tag="xq",
                                      name=f"xq{iqb + 1}")
                        dma_xq(iqb + 1, sl)
                        for dc in range(ndc):
                            fillers += q_unit(iqb + 1, dc, sl)
                    if iqb > 0:
                        for qc_l in range(qblk // P):
                            for nh in range(2):
                                fillers += o_unit((iqb - 1) * (qblk // P) + qc_l, nh)
                    fillers.reverse()  # pop() from the front

                    for pr in range(hpc // 2):
                        po = [pso.tile([dh + 1, qblk], F32, tag="po",
                                       name=f"po{iqb}_{pr}_{j}") for j in range(2)]
                        for kc in range(nkc):
                            last = kc == nkc - 1
                            ss = pss.tile([P, 2 * qblk], F32, tag="ss",
                                          name=f"ss{iqb}_{pr}_{kc}")
                            for hp in range(2):
                                mm(ss[:, hp * qblk:(hp + 1) * qblk],
                                   kt_t[pr][hp * dh:(hp + 1) * dh, kc * P:(kc + 1) * P],
                                   qt_t[pr][hp * dh:(hp + 1) * dh, q0:q0 + qblk],
                                   start=True, stop=True, tile_position=(hp * dh, 0))
                            pt = ptp.tile([P, 2 * qblk], mdt, tag="pt",
                                          name=f"pt{iqb}_{pr}_{kc}")
                            nc.scalar.activation(pt[:, :], ss[:, :], Exp,
                                                 bias=m_sb[:, kc:kc + 1])
                            for hp in range(2):
                                hh = 2 * pr + hp
                                mm(po[hp][:, :],
                                   v_t[kc][:, hh * (dh + 1):(hh + 1) * (dh + 1)],
                                   pt[:, hp * qblk:(hp + 1) * qblk],
                                   start=(kc == 0), stop=last)
                            # no pops in the last kc slot: keeps the queues
                            # clear for the PSUM-freeing po drains
                            npop = 0 if kc >= nkc - 1 else (3 if kc < 2 else 2)
                            hold = 8 if iqb == nqb - 1 else 0
                            for _ in range(npop):
                                if len(fillers) > hold:
                                    fillers.pop()()
                        # free the PSUM pair fast: the po halves drain on two
                        # engines at once; the denominator rows share one
                        # [1, 2*qblk] base-partition-0 tile (custom DVE ops
                        # mis-read inputs at a nonzero base partition) so a
                        # single wide reciprocal covers both heads.
                        po_sb = [None, None]
                        dr = rcp.tile([1, 2 * qblk], F32, tag="d",
                                      name=f"d{iqb}_{pr}")

                        def cp_d(hp, eng):
                            ecopy(eng, dr[0:1, hp * qblk:(hp + 1) * qblk],
                                  po[hp][dh:dh + 1, :])

                        def cp_po(hp, eng):
                            po_sb[hp] = rcp.tile([dh, qblk], F32, tag="posb",
                                                 name=f"posb{iqb}_{pr}_{hp}")
                            ecopy(eng, po_sb[hp][:, :], po[hp][0:dh, :])
                        last_pr = iqb == nqb - 1 and pr == hpc // 2 - 1
                        if last_pr:
                            # tail: shortest chain -- recip ASAP, and the muls
                            # read po straight from PSUM (no reuse pressure).
                            # ACT is idle after the last exp: use it for one
                            # denominator half.
                            cp_d(0, nc.vector)
                            cp_d(1, nc.scalar)
                        else:
                            # GPSIMD cannot read PSUM; the second denominator
                            # half rides the ACT pr-boundary bubble instead.
                            cp_po(0, nc.vector)
                            cp_po(1, nc.vector)
                            cp_d(0, nc.vector)
                            cp_d(1, nc.scalar)
                        rc = rcp.tile([1, 2 * qblk], F32, tag="rc",
                                      name=f"rc{iqb}_{pr}")
                        nc.vector.reciprocal_approx_fast(rc[:, :], dr[:, :])
                        for hp in range(2):
                            pb = pbp.tile([dh, qblk], F32, tag="pb",
                                          name=f"pb{iqb}_{pr}_{hp}")
                            nc.gpsimd.partition_broadcast(
                                pb[:, :], rc[0:1, hp * qblk:(hp + 1) * qblk],
                                channels=dh)
                            nc.vector.tensor_mul(
                                ot_t[pr][hp * dh:(hp + 1) * dh, q0:q0 + qblk],
                                po[hp][0:dh, :] if last_pr else po_sb[hp][:, :],
                                pb[:, :])
                    while fillers:
                        fillers.pop()()

                # phase D: O-projection of the last q-block, staggered so the
                # ndc-1 leading matmuls (which only need earlier head pairs'
                # ot) run while the final normalize chain completes.
                qc0 = (nqb - 1) * (qblk // P)
                units = [(qc0 + q, nh) for q in range(qblk // P) for nh in range(2)]
                dps = {}

                def d_head(i):
                    qc, nh = units[i]
                    ps = psf.tile([P, 512], F32, tag="f", name=f"fD{qc}_{nh}")
                    for dc in range(ndc - 1):
                        mm(ps[:, :], ot_t[dc][:, qc * P:(qc + 1) * P],
                           wo_t[:, dc, nh * 512:(nh + 1) * 512],
                           start=(dc == 0), stop=False)
                    dps[i] = ps

                def d_tail(i):
                    qc, nh = units[i]
                    ps = dps.pop(i)
                    mm(ps[:, :], ot_t[ndc - 1][:, qc * P:(qc + 1) * P],
                       wo_t[:, ndc - 1, nh * 512:(nh + 1) * 512],
                       start=False, stop=True)
                    ob = obp.tile([P, 512], F32, tag="ob", name=f"obD{qc}_{nh}")
                    ecopy(nc.scalar if i % 2 else nc.vector, ob[:, :], ps[:, :])
                    deng = (nc.sync, nc.scalar, nc.gpsimd)[i % 3]
                    deng.dma_start(out_d[qc * P:(qc + 1) * P,
                                         nh * 512:(nh + 1) * 512], ob[:, :])

                d_head(0)
                d_head(1)
                for i in range(len(units)):
                    d_tail(i)
                    if i + 2 < len(units):
                        d_head(i + 2)
    nc.finalize()
    return nc


def pack_w(w):
    """[C*128, N] -> [128, C*N]: partition p holds rows p, 128+p, ..."""
    c = w.shape[0] // P
    return np.ascontiguousarray(
        w.reshape(c, P, w.shape[1]).transpose(1, 0, 2).reshape(P, -1))


def pack_x(x, sw):
    """[1024, S] -> [128, S//sw, 8, sw] flattened: slab-major, 8 d-chunks."""
    nslb = x.shape[1] // sw
    return np.ascontiguousarray(
        x.reshape(8, P, nslb, sw).transpose(1, 2, 0, 3).reshape(P, -1))


def make_in_maps(queries, keys, values, mask, wq, bq, wk, bk, wv, bv, wo, bo,
                 nkc, with_bias=False):
    """Host-side shard prep. Core c -> (batch c//2, head-half c%2)."""
    scale = 1.0 / np.sqrt(np.float32(DH))
    sk = nkc * P
    qf = np.asarray(queries, np.float32)
    kf = np.asarray(keys, np.float32)
    vf = np.asarray(values, np.float32)
    wq_s = (np.asarray(wq, np.float32) * scale).astype(NP_BF16)
    wk_f = np.asarray(wk, np.float32).astype(NP_BF16)
    wv_f = np.asarray(wv, np.float32).astype(NP_BF16)
    wo_f = np.asarray(wo, np.float32).astype(NP_BF16)

    # per-batch key compression
    xkt_b, xvt_b, mrow_b = [], [], []
    for b in range(B):
        idx = np.flatnonzero(np.asarray(mask[b, 0, 0, :]) == 0)
        n = len(idx)
        assert 0 < n <= sk, (n, sk)
        kc_ = np.zeros((D, sk), NP_BF16)
        vc_ = np.zeros((D, sk), NP_BF16)
        kc_[:, :n] = kf[b, idx, :].T.astype(NP_BF16)
        vc_[:, :n] = vf[b, idx, :].T.astype(NP_BF16)
        m = np.full(sk, -1e9, np.float32)
        m[:n] = 0.0
        xkt_b.append(pack_x(kc_, 384))
        xvt_b.append(pack_x(vc_, 384))
        mrow_b.append(np.ascontiguousarray(m.reshape(nkc, P).T))

    in_maps = []
    for c in range(NCORES):
        b, hh = divmod(c, 2)
        ds = slice(hh * DPC, (hh + 1) * DPC)
        im = {
            "vones": np.ones((P, HPC), NP_BF16),
            "xqt": pack_x(qf[b].T.astype(NP_BF16), 512),
            "xkt": xkt_b[b],
            "xvt": xvt_b[b],
            "wq": pack_w(wq_s[:, ds]),
            "wk": pack_w(wk_f[:, ds]),
            "wv": pack_w(wv_f[:, ds]),
            "wo": pack_w(wo_f[ds, :]),
            "mrow": mrow_b[b],
        }
        if with_bias:
            im["ones"] = np.ones((1, 512), NP_BF16)
            im["biases"] = np.concatenate([
                np.asarray(bq, np.float32)[ds] * scale,
                np.asarray(bk, np.float32)[ds],
                np.asarray(bv, np.float32)[ds]]).astype(NP_BF16).reshape(1, 3 * DPC)
        in_maps.append(im)
    return in_maps


_CACHE = {}


def kernel(queries, keys, values, mask, wq, bq, wk, bk, wv, bv, wo, bo,
           _trace=False):
    with_bias = any(np.any(np.asarray(x)) for x in (bq, bk, bv))
    mask_np = np.asarray(mask)
    max_kept = max(int((mask_np[b, 0, 0, :] == 0).sum()) for b in range(B))
    # pad kept keys to a multiple of 384 (the K/V-proj slab width)
    nkc = -(-max(1, -(-max_kept // P)) // 3) * 3
    key = ("nc", nkc, with_bias)
    if key not in _CACHE:
        _CACHE[key] = build_nc(nkc, with_bias=with_bias)
    nc = _CACHE[key]
    in_maps = make_in_maps(queries, keys, values, mask, wq, bq, wk, bk,
                           wv, bv, wo, bo, nkc, with_bias=with_bias)
    res = run_bass_kernel_spmd(nc, in_maps, list(range(NCORES)), trace=_trace)
    out = np.empty((B, S, D), np.float32)
    for b in range(B):
        out[b] = res.results[2 * b]["out"] + res.results[2 * b + 1]["out"]
    if np.any(np.asarray(bo)):
        out += np.asarray(bo, np.float32)
    if _trace:
        return out, res
    return out
